# revision 14
# baseline (speedup 1.0000x reference)
"""DistMult edge scoring on 8 Trainium2 NeuronCores.

score[e] = sum_d node_emb[src[e], d] * rel_emb[e, d] * node_emb[dst[e], d]

Strategy (data-parallel over edges, per the sharding hint):
  - Edges sharded contiguously across 8 cores (125k/core, padded to whole
    128x32 tiles). Node table replicated per core in HBM.
  - Gather: gpsimd dma_gather is descriptor-count-bound (~8 ns/descriptor,
    measured), descriptor SIZE is nearly free, and int16 indices cap the
    addressable range at 32767. So the table is viewed as 25000 groups of
    4 rows (1 KB each): one descriptor fetches an edge's whole 4-row group
    (group id = node >> 2 < 25000 fits int16), and DVE selects the right
    row with host-precomputed 0/1 masks (node & 3). One descriptor per
    edge per direction - 4x fewer than any windowed row-gather scheme.
  - Per tile of 4096 edges: 8 gathers (2 dirs x 4 chunks of 1024, the HW
    per-instruction index cap), one rel load, DVE mask-select + product +
    segmented reduce, scores out.
  - Raw bacc with a manually pipelined 2-slot schedule (Tile does not
    semaphorize InstDMAGatherAnt).
"""

import numpy as np

import concourse.bacc as bacc
import concourse.bass as bass
import concourse.mybir as mybir
from concourse import library_config
from concourse.bass_utils import run_bass_kernel_spmd

N_NODES = 100000
DIM = 64
N_EDGES = 1000000
N_CORES = 8

P = 128
K = 32
TILE = P * K                      # 4096 edges per tile
EPC = N_EDGES // N_CORES          # 125000
NT = -(-EPC // TILE)              # 31
EPAD = NT * TILE                  # 126976

GR = 4                            # rows per gather group
NGRP = N_NODES // GR              # 25000 groups, fits int16
CHUNK = 1024                      # HW cap on num_idxs per dma_gather
CPT = TILE // CHUNK               # 4 chunks per tile
QC = CHUNK // P                   # 8 free slots per chunk

IDXF = TILE // 16                 # 256 int16s per partition per direction
F32 = mybir.dt.float32

_cache = {}


def _build_program():
    if "nc" in _cache:
        return _cache["nc"]

    nc = bacc.Bacc(
        "TRN2",
        target_bir_lowering=False,
        debug=False,
        enable_asserts=False,
        num_devices=N_CORES,
    )
    table = nc.dram_tensor("table", [NGRP, GR * DIM], F32, kind="ExternalInput")
    idx_h = nc.dram_tensor(
        "idx", [NT, P, 2, IDXF], mybir.dt.int16, kind="ExternalInput"
    )
    rel_h = nc.dram_tensor("rel", [NT, P, K, DIM], F32, kind="ExternalInput")
    msk_h = nc.dram_tensor("msk", [NT, P, 2, GR, K], F32, kind="ExternalInput")
    out_h = nc.dram_tensor("score", [NT, P, K], F32, kind="ExternalOutput")

    NB = 2     # gather-buffer pipeline slots
    NB_IO = 4  # idx/rel/msk prefetch slots
    NOPS = 17  # DVE ops per tile

    with (
        nc.Block() as block,
        nc.sbuf_tensor("gbuf", [P, NB, 2, CPT, QC, GR * DIM], F32) as gbuf,
        nc.sbuf_tensor("relb", [P, NB_IO, K, DIM], F32) as relb,
        nc.sbuf_tensor("mskb", [P, NB_IO, 2, GR, K], F32) as mskb,
        nc.sbuf_tensor("idxb", [P, NB_IO, 2, IDXF], mybir.dt.int16) as idxb,
        nc.sbuf_tensor("hbuf", [P, K, DIM], F32) as hbuf,
        nc.sbuf_tensor("tbuf", [P, K, DIM], F32) as tbuf,
        nc.sbuf_tensor("tmpb", [P, K, DIM], F32) as tmpb,
        nc.sbuf_tensor("sbuf_s", [P, NB, K], F32) as sb_s,
        nc.semaphore("s_idx") as s_idx,
        nc.semaphore("s_rel") as s_rel,
        nc.semaphore("s_msk") as s_msk,
        nc.semaphore("s_g") as s_g,
        nc.semaphore("s_vc") as s_vc,
        nc.semaphore("s_out") as s_out,
    ):

        @block.sync
        def _(sp: bass.BassEngine):
            # pure prefetcher: never gated on the compute chain beyond
            # slot reuse (NB_IO slots deep)
            for t in range(NT):
                s = t % NB_IO
                if t >= 1:
                    # order completions: sem count N must imply tiles 0..N-1
                    # are actually resident (DMAs can finish out of order)
                    sp.wait_ge(s_idx, 16 * t)
                    sp.wait_ge(s_rel, 16 * t)
                    sp.wait_ge(s_msk, 16 * t)
                if t >= NB_IO:
                    # idx slot free once tile t-NB_IO's gathers retired
                    sp.wait_ge(s_g, 128 * (t - NB_IO + 1))
                sp.dma_start(out=idxb[:, s], in_=idx_h[t]).then_inc(s_idx, 16)
                if t >= NB_IO:
                    # rel/msk slots consumed by DVE of tile t-NB_IO
                    sp.wait_ge(s_vc, NOPS * (t - NB_IO + 1))
                sp.dma_start(out=relb[:, s], in_=rel_h[t]).then_inc(s_rel, 16)
                sp.dma_start(out=mskb[:, s], in_=msk_h[t]).then_inc(s_msk, 16)

        @block.scalar
        def _(sc: bass.BassEngine):
            # out-stores, decoupled from the prefetch stream
            for t in range(NT):
                sc.wait_ge(s_vc, NOPS * (t + 1))
                if t >= 1:
                    sc.wait_ge(s_out, 16 * t)
                sc.dma_start(
                    out=out_h[t], in_=sb_s[:, t % NB]
                ).then_inc(s_out, 16)
            sc.wait_ge(s_out, 16 * NT)

        @block.gpsimd
        def _(gp: bass.BassGpSimd):
            gp.load_library(library_config.mlp)
            for t in range(NT):
                s = t % NB
                gp.wait_ge(s_idx, 16 * (t + 1))
                if t >= 1:
                    # order gather completions across tiles: count 128*(t+1)
                    # must imply tiles 0..t fully landed
                    gp.wait_ge(s_g, 128 * t)
                if t >= NB:
                    # gather buffers of tile t-NB consumed by DVE
                    gp.wait_ge(s_vc, NOPS * (t - NB + 1))
                for d in range(2):
                    for c in range(CPT):
                        gp.dma_gather(
                            gbuf[:, s, d, c],
                            table[:],
                            idxb[:, t % NB_IO, d, c * 64 : (c + 1) * 64],
                            CHUNK,
                            CHUNK,
                            GR * DIM,
                            elem_step=GR * DIM,
                        ).then_inc(s_g, 16)

        @block.vector
        def _(v: bass.BassEngine):
            mult = mybir.AluOpType.mult
            add = mybir.AluOpType.add
            for t in range(NT):
                s = t % NB
                v.wait_ge(s_g, 128 * (t + 1))
                v.wait_ge(s_rel, 16 * (t + 1))
                v.wait_ge(s_msk, 16 * (t + 1))
                if t >= NB:
                    v.wait_ge(s_out, 16 * (t - NB + 1))
                if t >= 1:
                    # hbuf/tbuf/tmpb WAR vs previous tile's chain
                    v.wait_ge(s_vc, NOPS * t)
                i = NOPS * t

                def op(instr):
                    nonlocal i
                    i += 1
                    instr.then_inc(s_vc, 1)

                def wait():
                    v.wait_ge(s_vc, i)

                for d, dst in ((0, hbuf), (1, tbuf)):
                    # g viewed [P, K, GR*DIM]: (c, q) -> k = c*QC + q
                    g = gbuf[:, s, d].rearrange("p c q e -> p (c q) e")
                    for r in range(GR):
                        m = mskb[:, t % NB_IO, d, r].to_broadcast([P, K, DIM])
                        gsl = g[:, :, r * DIM : (r + 1) * DIM]
                        if r == 0:
                            op(v.tensor_tensor(out=dst[:], in0=gsl, in1=m, op=mult))
                        else:
                            wait()
                            op(
                                v.tensor_tensor(
                                    out=tmpb[:], in0=gsl, in1=m, op=mult
                                )
                            )
                            wait()
                            op(
                                v.tensor_tensor(
                                    out=dst[:], in0=dst[:], in1=tmpb[:], op=add
                                )
                            )
                wait()
                op(
                    v.tensor_tensor(
                        out=hbuf[:], in0=hbuf[:], in1=relb[:, t % NB_IO], op=mult
                    )
                )
                wait()
                op(
                    v.tensor_tensor(
                        out=hbuf[:], in0=hbuf[:], in1=tbuf[:], op=mult
                    )
                )
                wait()
                v.tensor_reduce(
                    out=sb_s[:, s],
                    in_=hbuf[:],
                    axis=mybir.AxisListType.X,
                    op=add,
                ).then_inc(s_vc, 1)

    nc.compile()
    _cache["nc"] = nc
    return nc


def _prep_idx(idx_global):
    """(EPAD,) node ids -> wrapped int16 group indices [NT, P, IDXF]."""
    g = (idx_global.reshape(NT, TILE) >> 2).astype(np.int16)
    # wrap: index j -> [j % 16, j // 16], replicated across 8 partition groups
    wr = g.reshape(NT, IDXF, 16).swapaxes(1, 2)  # [NT, 16, IDXF]
    return np.broadcast_to(wr[:, None, :, :], (NT, 8, 16, IDXF)).reshape(
        NT, P, IDXF
    )


def _prep_msk(idx_global):
    """(EPAD,) node ids -> 0/1 row-select masks [NT, P, GR, K]."""
    sub = (idx_global.reshape(NT, K, P) & 3).astype(np.int8)  # [t, k, p]
    m = (sub[:, None, :, :] == np.arange(GR, dtype=np.int8)[None, :, None, None])
    # [NT, GR, K, P] -> [NT, P, GR, K]
    return np.ascontiguousarray(m.transpose(0, 3, 1, 2)).astype(np.float32)


def _shard_inputs(node_emb, rel_emb, src, dst):
    node_emb = np.asarray(node_emb, dtype=np.float32)
    rel_emb = np.asarray(rel_emb, dtype=np.float32)
    src = np.asarray(src, dtype=np.int64)
    dst = np.asarray(dst, dtype=np.int64)

    table = np.ascontiguousarray(node_emb.reshape(NGRP, GR * DIM))

    in_maps = []
    for c in range(N_CORES):
        sl = slice(c * EPC, (c + 1) * EPC)
        src_c = np.zeros(EPAD, np.int64)
        dst_c = np.zeros(EPAD, np.int64)
        rel_c = np.zeros((EPAD, DIM), np.float32)
        src_c[:EPC] = src[sl]
        dst_c[:EPC] = dst[sl]
        rel_c[:EPC] = rel_emb[sl]

        idx = np.stack([_prep_idx(src_c), _prep_idx(dst_c)], axis=2)
        # [NT, P, 2, IDXF]
        msk = np.stack([_prep_msk(src_c), _prep_msk(dst_c)], axis=2)
        # [NT, P, 2, GR, K]
        # edge j at [p = j % 128, k = j // 128] -> rel[t, p, k]
        rel_t = np.ascontiguousarray(
            rel_c.reshape(NT, K, P, DIM).swapaxes(1, 2)
        )
        in_maps.append(
            {
                "table": table,
                "idx": np.ascontiguousarray(idx),
                "rel": rel_t,
                "msk": msk,
            }
        )
    return in_maps


def run_on_hw(node_emb, rel_emb, src, dst, **spmd_kwargs):
    nc = _build_program()
    in_maps = _shard_inputs(node_emb, rel_emb, src, dst)
    res = run_bass_kernel_spmd(nc, in_maps, list(range(N_CORES)), **spmd_kwargs)
    parts = [
        np.asarray(res.results[c]["score"])
        .transpose(0, 2, 1)
        .reshape(EPAD)[:EPC]
        for c in range(N_CORES)
    ]
    return np.concatenate(parts), res


def kernel(node_emb, rel_emb, src, dst):
    scores, _ = run_on_hw(node_emb, rel_emb, src, dst)
    return scores


# revision 17
# speedup vs baseline: 1.0940x; 1.0940x over previous
"""DistMult edge scoring on 8 Trainium2 NeuronCores.

score[e] = sum_d node_emb[src[e], d] * rel_emb[e, d] * node_emb[dst[e], d]

Strategy (data-parallel over edges, per the sharding hint):
  - Edges sharded contiguously across 8 cores (125k/core, padded to whole
    128x32 tiles). Node table replicated per core in HBM.
  - Gather: gpsimd dma_gather is descriptor-count-bound (~8 ns/descriptor,
    measured), descriptor SIZE is nearly free, and int16 indices cap the
    addressable range at 32767. So the table is viewed as 25000 groups of
    4 rows (1 KB each): one descriptor fetches an edge's whole 4-row group
    (group id = node >> 2 < 25000 fits int16), and DVE selects the right
    row with host-precomputed 0/1 masks (node & 3). One descriptor per
    edge per direction - 4x fewer than any windowed row-gather scheme.
  - Per tile of 4096 edges: 8 gathers (2 dirs x 4 chunks of 1024, the HW
    per-instruction index cap), one rel load, DVE mask-select + product +
    segmented reduce, scores out.
  - Raw bacc with a manually pipelined 2-slot schedule (Tile does not
    semaphorize InstDMAGatherAnt).
"""

import numpy as np

import concourse.bacc as bacc
import concourse.bass as bass
import concourse.mybir as mybir
from concourse import library_config
from concourse.bass_utils import run_bass_kernel_spmd

N_NODES = 100000
DIM = 64
N_EDGES = 1000000
N_CORES = 8

P = 128
K = 32
TILE = P * K                      # 4096 edges per tile
EPC = N_EDGES // N_CORES          # 125000
NT = -(-EPC // TILE)              # 31
EPAD = NT * TILE                  # 126976

GR = 4                            # rows per gather group
NGRP = N_NODES // GR              # 25000 groups, fits int16
CHUNK = 1024                      # HW cap on num_idxs per dma_gather
CPT = TILE // CHUNK               # 4 chunks per tile
QC = CHUNK // P                   # 8 free slots per chunk

IDXF = TILE // 16                 # 256 int16s per partition per direction
F32 = mybir.dt.float32

_cache = {}


def _chunk_valid(t, c):
    """Valid (non-pad) edge count in tile t, chunk c; pad idx are trailing
    -1s skipped by num_idxs_reg semantics. All-pad chunks keep one valid
    index (host forces slot 0 to group 0) so the DMA completion fires."""
    lo = t * TILE + c * CHUNK
    return int(max(1, min(CHUNK, EPC - lo)))


def _build_program():
    if "nc" in _cache:
        return _cache["nc"]

    nc = bacc.Bacc(
        "TRN2",
        target_bir_lowering=False,
        debug=False,
        enable_asserts=False,
        num_devices=N_CORES,
    )
    table = nc.dram_tensor("table", [NGRP, GR * DIM], F32, kind="ExternalInput")
    idx_h = nc.dram_tensor(
        "idx", [NT, P, 2, IDXF], mybir.dt.int16, kind="ExternalInput"
    )
    rel_h = nc.dram_tensor("rel", [NT, P, K, DIM], F32, kind="ExternalInput")
    msk_h = nc.dram_tensor("msk", [NT, P, 2, GR, K], F32, kind="ExternalInput")
    out_h = nc.dram_tensor("score", [NT, P, K], F32, kind="ExternalOutput")

    NB = 2     # gather-buffer pipeline slots
    NB_IO = 4  # idx/rel/msk prefetch slots
    NOPS = 17  # DVE ops per tile

    with (
        nc.Block() as block,
        nc.sbuf_tensor("gbuf", [P, NB, 2, CPT, QC, GR * DIM], F32) as gbuf,
        nc.sbuf_tensor("relb", [P, NB_IO, K, DIM], F32) as relb,
        nc.sbuf_tensor("mskb", [P, NB_IO, 2, GR, K], F32) as mskb,
        nc.sbuf_tensor("idxb", [P, NB_IO, 2, IDXF], mybir.dt.int16) as idxb,
        nc.sbuf_tensor("hbuf", [P, K, DIM], F32) as hbuf,
        nc.sbuf_tensor("tbuf", [P, K, DIM], F32) as tbuf,
        nc.sbuf_tensor("tmpb", [P, K, DIM], F32) as tmpb,
        nc.sbuf_tensor("sbuf_s", [P, NB, K], F32) as sb_s,
        nc.semaphore("s_idx") as s_idx,
        nc.semaphore("s_rel") as s_rel,
        nc.semaphore("s_msk") as s_msk,
        nc.semaphore("s_g") as s_g,
        nc.semaphore("s_g2") as s_g2,
        nc.semaphore("s_vc") as s_vc,
        nc.semaphore("s_out") as s_out,
    ):

        @block.sync
        def _(sp: bass.BassEngine):
            # pure prefetcher: never gated on the compute chain beyond
            # slot reuse (NB_IO slots deep)
            for t in range(NT):
                s = t % NB_IO
                if t >= 1:
                    # order completions: sem count N must imply tiles 0..N-1
                    # are actually resident (DMAs can finish out of order)
                    sp.wait_ge(s_idx, 16 * t)
                    sp.wait_ge(s_rel, 16 * t)
                    sp.wait_ge(s_msk, 16 * t)
                if t >= NB_IO:
                    # idx slot free once tile t-NB_IO's gathers retired
                    tt = t - NB_IO
                    sp.wait_ge(s_g if tt % 2 == 0 else s_g2, 128 * (tt // 2 + 1))
                sp.dma_start(out=idxb[:, s], in_=idx_h[t]).then_inc(s_idx, 16)
                if t >= NB_IO:
                    # rel/msk slots consumed by DVE of tile t-NB_IO
                    sp.wait_ge(s_vc, NOPS * (t - NB_IO + 1))
                sp.dma_start(out=relb[:, s], in_=rel_h[t]).then_inc(s_rel, 16)
                sp.dma_start(out=mskb[:, s], in_=msk_h[t]).then_inc(s_msk, 16)

        @block.scalar
        def _(sc: bass.BassEngine):
            # out-stores, decoupled from the prefetch stream
            for t in range(NT):
                sc.wait_ge(s_vc, NOPS * (t + 1))
                if t >= 1:
                    sc.wait_ge(s_out, 16 * t)
                sc.dma_start(
                    out=out_h[t], in_=sb_s[:, t % NB]
                ).then_inc(s_out, 16)
            sc.wait_ge(s_out, 16 * NT)

        @block.gpsimd
        def _(gp: bass.BassGpSimd):
            gp.load_library(library_config.mlp)
            for t in range(NT):
                s = t % NB
                gp.wait_ge(s_idx, 16 * (t + 1))
                if t >= NB:
                    # gather buffers of tile t-NB consumed by DVE
                    gp.wait_ge(s_vc, NOPS * (t - NB + 1))
                for d in range(2):
                    for c in range(CPT):
                        gp.dma_gather(
                            gbuf[:, s, d, c],
                            table[:],
                            idxb[:, t % NB_IO, d, c * 64 : (c + 1) * 64],
                            CHUNK,
                            _chunk_valid(t, c),
                            GR * DIM,
                            elem_step=GR * DIM,
                            single_packet=False,
                        ).then_inc(s_g if t % 2 == 0 else s_g2, 16)

        @block.vector
        def _(v: bass.BassEngine):
            mult = mybir.AluOpType.mult
            add = mybir.AluOpType.add
            for t in range(NT):
                s = t % NB
                v.wait_ge(s_g if t % 2 == 0 else s_g2, 128 * (t // 2 + 1))
                v.wait_ge(s_rel, 16 * (t + 1))
                v.wait_ge(s_msk, 16 * (t + 1))
                if t >= NB:
                    v.wait_ge(s_out, 16 * (t - NB + 1))
                if t >= 1:
                    # hbuf/tbuf/tmpb WAR vs previous tile's chain
                    v.wait_ge(s_vc, NOPS * t)
                i = NOPS * t

                def op(instr):
                    nonlocal i
                    i += 1
                    instr.then_inc(s_vc, 1)

                def wait():
                    v.wait_ge(s_vc, i)

                for d, dst in ((0, hbuf), (1, tbuf)):
                    # g viewed [P, K, GR*DIM]: (c, q) -> k = c*QC + q
                    g = gbuf[:, s, d].rearrange("p c q e -> p (c q) e")
                    for r in range(GR):
                        m = mskb[:, t % NB_IO, d, r].to_broadcast([P, K, DIM])
                        gsl = g[:, :, r * DIM : (r + 1) * DIM]
                        if r == 0:
                            op(v.tensor_tensor(out=dst[:], in0=gsl, in1=m, op=mult))
                        else:
                            wait()
                            op(
                                v.tensor_tensor(
                                    out=tmpb[:], in0=gsl, in1=m, op=mult
                                )
                            )
                            wait()
                            op(
                                v.tensor_tensor(
                                    out=dst[:], in0=dst[:], in1=tmpb[:], op=add
                                )
                            )
                wait()
                op(
                    v.tensor_tensor(
                        out=hbuf[:], in0=hbuf[:], in1=relb[:, t % NB_IO], op=mult
                    )
                )
                wait()
                op(
                    v.tensor_tensor(
                        out=hbuf[:], in0=hbuf[:], in1=tbuf[:], op=mult
                    )
                )
                wait()
                v.tensor_reduce(
                    out=sb_s[:, s],
                    in_=hbuf[:],
                    axis=mybir.AxisListType.X,
                    op=add,
                ).then_inc(s_vc, 1)

    nc.compile()
    _cache["nc"] = nc
    return nc


def _prep_idx(idx_global):
    """(EPAD,) node ids -> wrapped int16 group indices [NT, P, IDXF].
    Pad edges (index < 0) stay -1 and are skipped via num_idxs_reg."""
    n = idx_global.reshape(NT, TILE)
    g = np.where(n >= 0, n >> 2, -1).astype(np.int16)
    for t in range(NT):
        for c in range(CPT):
            if t * TILE + c * CHUNK >= EPC:
                g[t, c * CHUNK] = 0  # keep >=1 valid idx per chunk
    # wrap: index j -> [j % 16, j // 16], replicated across 8 partition groups
    wr = g.reshape(NT, IDXF, 16).swapaxes(1, 2)  # [NT, 16, IDXF]
    return np.broadcast_to(wr[:, None, :, :], (NT, 8, 16, IDXF)).reshape(
        NT, P, IDXF
    )


def _prep_msk(idx_global):
    """(EPAD,) node ids -> 0/1 row-select masks [NT, P, GR, K]."""
    sub = (np.maximum(idx_global.reshape(NT, K, P), 0) & 3).astype(np.int8)
    m = (sub[:, None, :, :] == np.arange(GR, dtype=np.int8)[None, :, None, None])
    # [NT, GR, K, P] -> [NT, P, GR, K]
    return np.ascontiguousarray(m.transpose(0, 3, 1, 2)).astype(np.float32)


def _shard_inputs(node_emb, rel_emb, src, dst):
    node_emb = np.asarray(node_emb, dtype=np.float32)
    rel_emb = np.asarray(rel_emb, dtype=np.float32)
    src = np.asarray(src, dtype=np.int64)
    dst = np.asarray(dst, dtype=np.int64)

    table = np.ascontiguousarray(node_emb.reshape(NGRP, GR * DIM))

    in_maps = []
    for c in range(N_CORES):
        sl = slice(c * EPC, (c + 1) * EPC)
        src_c = np.full(EPAD, -1, np.int64)
        dst_c = np.full(EPAD, -1, np.int64)
        rel_c = np.zeros((EPAD, DIM), np.float32)
        src_c[:EPC] = src[sl]
        dst_c[:EPC] = dst[sl]
        rel_c[:EPC] = rel_emb[sl]

        idx = np.stack([_prep_idx(src_c), _prep_idx(dst_c)], axis=2)
        # [NT, P, 2, IDXF]
        msk = np.stack([_prep_msk(src_c), _prep_msk(dst_c)], axis=2)
        # [NT, P, 2, GR, K]
        # edge j at [p = j % 128, k = j // 128] -> rel[t, p, k]
        rel_t = np.ascontiguousarray(
            rel_c.reshape(NT, K, P, DIM).swapaxes(1, 2)
        )
        in_maps.append(
            {
                "table": table,
                "idx": np.ascontiguousarray(idx),
                "rel": rel_t,
                "msk": msk,
            }
        )
    return in_maps


def run_on_hw(node_emb, rel_emb, src, dst, **spmd_kwargs):
    nc = _build_program()
    in_maps = _shard_inputs(node_emb, rel_emb, src, dst)
    res = run_bass_kernel_spmd(nc, in_maps, list(range(N_CORES)), **spmd_kwargs)
    parts = [
        np.asarray(res.results[c]["score"])
        .transpose(0, 2, 1)
        .reshape(EPAD)[:EPC]
        for c in range(N_CORES)
    ]
    return np.concatenate(parts), res


def kernel(node_emb, rel_emb, src, dst):
    scores, _ = run_on_hw(node_emb, rel_emb, src, dst)
    return scores


# revision 18
# speedup vs baseline: 1.1776x; 1.0765x over previous
"""DistMult edge scoring on 8 Trainium2 NeuronCores.

score[e] = sum_d node_emb[src[e], d] * rel_emb[e, d] * node_emb[dst[e], d]

Strategy (data-parallel over edges, per the sharding hint):
  - Edges sharded contiguously across 8 cores (125k/core, padded to whole
    128x32 tiles). Node table replicated per core in HBM.
  - Gather: gpsimd dma_gather is descriptor-count-bound (~8 ns/descriptor,
    measured), descriptor SIZE is nearly free, and int16 indices cap the
    addressable range at 32767. So the table is viewed as 25000 groups of
    4 rows (1 KB each): one descriptor fetches an edge's whole 4-row group
    (group id = node >> 2 < 25000 fits int16), and DVE selects the right
    row with host-precomputed 0/1 masks (node & 3). One descriptor per
    edge per direction - 4x fewer than any windowed row-gather scheme.
  - Per tile of 4096 edges: 8 gathers (2 dirs x 4 chunks of 1024, the HW
    per-instruction index cap), one rel load, DVE mask-select + product +
    segmented reduce, scores out.
  - Raw bacc with a manually pipelined 2-slot schedule (Tile does not
    semaphorize InstDMAGatherAnt).
"""

import numpy as np

import concourse.bacc as bacc
import concourse.bass as bass
import concourse.mybir as mybir
from concourse import library_config
from concourse.bass_utils import run_bass_kernel_spmd

N_NODES = 100000
DIM = 64
N_EDGES = 1000000
N_CORES = 8

P = 128
K = 32
TILE = P * K                      # 4096 edges per tile
EPC = N_EDGES // N_CORES          # 125000
NT = -(-EPC // TILE)              # 31
EPAD = NT * TILE                  # 126976

GR = 4                            # rows per gather group
NGRP = N_NODES // GR              # 25000 groups, fits int16
CHUNK = TILE                      # idx per dma_gather (multi-packet mode)
CPT = TILE // CHUNK               # 1 chunk per tile
QC = CHUNK // P                   # 32 free slots per chunk

IDXF = TILE // 16                 # 256 int16s per partition per direction
F32 = mybir.dt.float32

_cache = {}


def _chunk_valid(t, c):
    """Valid (non-pad) edge count in tile t, chunk c; pad idx are trailing
    -1s skipped by num_idxs_reg semantics. All-pad chunks keep one valid
    index (host forces slot 0 to group 0) so the DMA completion fires."""
    lo = t * TILE + c * CHUNK
    return int(max(1, min(CHUNK, EPC - lo)))


def _build_program():
    if "nc" in _cache:
        return _cache["nc"]

    nc = bacc.Bacc(
        "TRN2",
        target_bir_lowering=False,
        debug=False,
        enable_asserts=False,
        num_devices=N_CORES,
    )
    table = nc.dram_tensor("table", [NGRP, GR * DIM], F32, kind="ExternalInput")
    idx_h = nc.dram_tensor(
        "idx", [NT, P, 2, IDXF], mybir.dt.int16, kind="ExternalInput"
    )
    rel_h = nc.dram_tensor("rel", [NT, P, K, DIM], F32, kind="ExternalInput")
    msk_h = nc.dram_tensor("msk", [NT, P, 2, GR, K], F32, kind="ExternalInput")
    out_h = nc.dram_tensor("score", [NT, P, K], F32, kind="ExternalOutput")

    NB = 2     # gather-buffer pipeline slots
    NB_IO = 4  # idx/rel/msk prefetch slots
    NOPS = 17  # DVE ops per tile

    with (
        nc.Block() as block,
        nc.sbuf_tensor("gbuf", [P, NB, 2, CPT, QC, GR * DIM], F32) as gbuf,
        nc.sbuf_tensor("relb", [P, NB_IO, K, DIM], F32) as relb,
        nc.sbuf_tensor("mskb", [P, NB_IO, 2, GR, K], F32) as mskb,
        nc.sbuf_tensor("idxb", [P, NB_IO, 2, IDXF], mybir.dt.int16) as idxb,
        nc.sbuf_tensor("hbuf", [P, K, DIM], F32) as hbuf,
        nc.sbuf_tensor("tbuf", [P, K, DIM], F32) as tbuf,
        nc.sbuf_tensor("tmpb", [P, K, DIM], F32) as tmpb,
        nc.sbuf_tensor("sbuf_s", [P, NB, K], F32) as sb_s,
        nc.semaphore("s_idx") as s_idx,
        nc.semaphore("s_rel") as s_rel,
        nc.semaphore("s_msk") as s_msk,
        nc.semaphore("s_g") as s_g,
        nc.semaphore("s_g2") as s_g2,
        nc.semaphore("s_vc") as s_vc,
        nc.semaphore("s_out") as s_out,
    ):

        @block.sync
        def _(sp: bass.BassEngine):
            # pure prefetcher: never gated on the compute chain beyond
            # slot reuse (NB_IO slots deep)
            for t in range(NT):
                s = t % NB_IO
                if t >= 1:
                    # order completions: sem count N must imply tiles 0..N-1
                    # are actually resident (DMAs can finish out of order)
                    sp.wait_ge(s_idx, 16 * t)
                    sp.wait_ge(s_rel, 16 * t)
                    sp.wait_ge(s_msk, 16 * t)
                if t >= NB_IO:
                    # idx slot free once tile t-NB_IO's gathers retired
                    tt = t - NB_IO
                    sp.wait_ge(s_g if tt % 2 == 0 else s_g2, 32 * (tt // 2 + 1))
                sp.dma_start(out=idxb[:, s], in_=idx_h[t]).then_inc(s_idx, 16)
                if t >= NB_IO:
                    # rel/msk slots consumed by DVE of tile t-NB_IO
                    sp.wait_ge(s_vc, NOPS * (t - NB_IO + 1))
                sp.dma_start(out=relb[:, s], in_=rel_h[t]).then_inc(s_rel, 16)
                sp.dma_start(out=mskb[:, s], in_=msk_h[t]).then_inc(s_msk, 16)

        @block.scalar
        def _(sc: bass.BassEngine):
            # out-stores, decoupled from the prefetch stream
            for t in range(NT):
                sc.wait_ge(s_vc, NOPS * (t + 1))
                if t >= 1:
                    sc.wait_ge(s_out, 16 * t)
                sc.dma_start(
                    out=out_h[t], in_=sb_s[:, t % NB]
                ).then_inc(s_out, 16)
            sc.wait_ge(s_out, 16 * NT)

        @block.gpsimd
        def _(gp: bass.BassGpSimd):
            gp.load_library(library_config.mlp)
            for t in range(NT):
                s = t % NB
                gp.wait_ge(s_idx, 16 * (t + 1))
                if t >= NB:
                    # gather buffers of tile t-NB consumed by DVE
                    gp.wait_ge(s_vc, NOPS * (t - NB + 1))
                for d in range(2):
                    gp.dma_gather(
                        gbuf[:, s, d, 0],
                        table[:],
                        idxb[:, t % NB_IO, d],
                        CHUNK,
                        _chunk_valid(t, 0),
                        GR * DIM,
                        elem_step=GR * DIM,
                        single_packet=False,
                    ).then_inc(s_g if t % 2 == 0 else s_g2, 16)

        @block.vector
        def _(v: bass.BassEngine):
            mult = mybir.AluOpType.mult
            add = mybir.AluOpType.add
            for t in range(NT):
                s = t % NB
                v.wait_ge(s_g if t % 2 == 0 else s_g2, 32 * (t // 2 + 1))
                v.wait_ge(s_rel, 16 * (t + 1))
                v.wait_ge(s_msk, 16 * (t + 1))
                if t >= NB:
                    v.wait_ge(s_out, 16 * (t - NB + 1))
                if t >= 1:
                    # hbuf/tbuf/tmpb WAR vs previous tile's chain
                    v.wait_ge(s_vc, NOPS * t)
                i = NOPS * t

                def op(instr):
                    nonlocal i
                    i += 1
                    instr.then_inc(s_vc, 1)

                def wait():
                    v.wait_ge(s_vc, i)

                for d, dst in ((0, hbuf), (1, tbuf)):
                    # g viewed [P, K, GR*DIM]: (c, q) -> k = c*QC + q
                    g = gbuf[:, s, d].rearrange("p c q e -> p (c q) e")
                    for r in range(GR):
                        m = mskb[:, t % NB_IO, d, r].to_broadcast([P, K, DIM])
                        gsl = g[:, :, r * DIM : (r + 1) * DIM]
                        if r == 0:
                            op(v.tensor_tensor(out=dst[:], in0=gsl, in1=m, op=mult))
                        else:
                            wait()
                            op(
                                v.tensor_tensor(
                                    out=tmpb[:], in0=gsl, in1=m, op=mult
                                )
                            )
                            wait()
                            op(
                                v.tensor_tensor(
                                    out=dst[:], in0=dst[:], in1=tmpb[:], op=add
                                )
                            )
                wait()
                op(
                    v.tensor_tensor(
                        out=hbuf[:], in0=hbuf[:], in1=relb[:, t % NB_IO], op=mult
                    )
                )
                wait()
                op(
                    v.tensor_tensor(
                        out=hbuf[:], in0=hbuf[:], in1=tbuf[:], op=mult
                    )
                )
                wait()
                v.tensor_reduce(
                    out=sb_s[:, s],
                    in_=hbuf[:],
                    axis=mybir.AxisListType.X,
                    op=add,
                ).then_inc(s_vc, 1)

    nc.compile()
    _cache["nc"] = nc
    return nc


def _prep_idx(idx_global):
    """(EPAD,) node ids -> wrapped int16 group indices [NT, P, IDXF].
    Pad edges (index < 0) stay -1 and are skipped via num_idxs_reg."""
    n = idx_global.reshape(NT, TILE)
    g = np.where(n >= 0, n >> 2, -1).astype(np.int16)
    for t in range(NT):
        for c in range(CPT):
            if t * TILE + c * CHUNK >= EPC:
                g[t, c * CHUNK] = 0  # keep >=1 valid idx per chunk
    # wrap: index j -> [j % 16, j // 16], replicated across 8 partition groups
    wr = g.reshape(NT, IDXF, 16).swapaxes(1, 2)  # [NT, 16, IDXF]
    return np.broadcast_to(wr[:, None, :, :], (NT, 8, 16, IDXF)).reshape(
        NT, P, IDXF
    )


def _prep_msk(idx_global):
    """(EPAD,) node ids -> 0/1 row-select masks [NT, P, GR, K]."""
    sub = (np.maximum(idx_global.reshape(NT, K, P), 0) & 3).astype(np.int8)
    m = (sub[:, None, :, :] == np.arange(GR, dtype=np.int8)[None, :, None, None])
    # [NT, GR, K, P] -> [NT, P, GR, K]
    return np.ascontiguousarray(m.transpose(0, 3, 1, 2)).astype(np.float32)


def _shard_inputs(node_emb, rel_emb, src, dst):
    node_emb = np.asarray(node_emb, dtype=np.float32)
    rel_emb = np.asarray(rel_emb, dtype=np.float32)
    src = np.asarray(src, dtype=np.int64)
    dst = np.asarray(dst, dtype=np.int64)

    table = np.ascontiguousarray(node_emb.reshape(NGRP, GR * DIM))

    in_maps = []
    for c in range(N_CORES):
        sl = slice(c * EPC, (c + 1) * EPC)
        src_c = np.full(EPAD, -1, np.int64)
        dst_c = np.full(EPAD, -1, np.int64)
        rel_c = np.zeros((EPAD, DIM), np.float32)
        src_c[:EPC] = src[sl]
        dst_c[:EPC] = dst[sl]
        rel_c[:EPC] = rel_emb[sl]

        idx = np.stack([_prep_idx(src_c), _prep_idx(dst_c)], axis=2)
        # [NT, P, 2, IDXF]
        msk = np.stack([_prep_msk(src_c), _prep_msk(dst_c)], axis=2)
        # [NT, P, 2, GR, K]
        # edge j at [p = j % 128, k = j // 128] -> rel[t, p, k]
        rel_t = np.ascontiguousarray(
            rel_c.reshape(NT, K, P, DIM).swapaxes(1, 2)
        )
        in_maps.append(
            {
                "table": table,
                "idx": np.ascontiguousarray(idx),
                "rel": rel_t,
                "msk": msk,
            }
        )
    return in_maps


def run_on_hw(node_emb, rel_emb, src, dst, **spmd_kwargs):
    nc = _build_program()
    in_maps = _shard_inputs(node_emb, rel_emb, src, dst)
    res = run_bass_kernel_spmd(nc, in_maps, list(range(N_CORES)), **spmd_kwargs)
    parts = [
        np.asarray(res.results[c]["score"])
        .transpose(0, 2, 1)
        .reshape(EPAD)[:EPC]
        for c in range(N_CORES)
    ]
    return np.concatenate(parts), res


def kernel(node_emb, rel_emb, src, dst):
    scores, _ = run_on_hw(node_emb, rel_emb, src, dst)
    return scores


# revision 20
# speedup vs baseline: 1.1827x; 1.0043x over previous
"""DistMult edge scoring on 8 Trainium2 NeuronCores.

score[e] = sum_d node_emb[src[e], d] * rel_emb[e, d] * node_emb[dst[e], d]

Strategy (data-parallel over edges, per the sharding hint):
  - Edges sharded contiguously across 8 cores (125k/core, padded to whole
    128x32 tiles). Node table replicated per core in HBM.
  - Gather: gpsimd dma_gather is descriptor-count-bound (~8 ns/descriptor,
    measured), descriptor SIZE is nearly free, and int16 indices cap the
    addressable range at 32767. So the table is viewed as 25000 groups of
    4 rows (1 KB each): one descriptor fetches an edge's whole 4-row group
    (group id = node >> 2 < 25000 fits int16), and DVE selects the right
    row with host-precomputed 0/1 masks (node & 3). One descriptor per
    edge per direction - 4x fewer than any windowed row-gather scheme.
  - Per tile of 4096 edges: 8 gathers (2 dirs x 4 chunks of 1024, the HW
    per-instruction index cap), one rel load, DVE mask-select + product +
    segmented reduce, scores out.
  - Raw bacc with a manually pipelined 2-slot schedule (Tile does not
    semaphorize InstDMAGatherAnt).
"""

import numpy as np

import concourse.bacc as bacc
import concourse.bass as bass
import concourse.mybir as mybir
from concourse import library_config
from concourse.bass_utils import run_bass_kernel_spmd

N_NODES = 100000
DIM = 64
N_EDGES = 1000000
N_CORES = 8

P = 128
K = 32
TILE = P * K                      # 4096 edges per tile
EPC = N_EDGES // N_CORES          # 125000
NT = -(-EPC // TILE)              # 31
EPAD = NT * TILE                  # 126976

GR = 4                            # rows per gather group
NGRP = N_NODES // GR              # 25000 groups, fits int16
CHUNK = TILE                      # idx per dma_gather (multi-packet mode)
CPT = TILE // CHUNK               # 1 chunk per tile
QC = CHUNK // P                   # 32 free slots per chunk

IDXF = TILE // 16                 # 256 int16s per partition per direction
F32 = mybir.dt.float32

_cache = {}


def _chunk_valid(t, c):
    """Valid (non-pad) edge count in tile t, chunk c; pad idx are trailing
    -1s skipped by num_idxs_reg semantics. All-pad chunks keep one valid
    index (host forces slot 0 to group 0) so the DMA completion fires."""
    lo = t * TILE + c * CHUNK
    return int(max(1, min(CHUNK, EPC - lo)))


def _build_program():
    if "nc" in _cache:
        return _cache["nc"]

    nc = bacc.Bacc(
        "TRN2",
        target_bir_lowering=False,
        debug=False,
        enable_asserts=False,
        num_devices=N_CORES,
    )
    table = nc.dram_tensor("table", [NGRP, GR * DIM], F32, kind="ExternalInput")
    idx_h = nc.dram_tensor(
        "idx", [NT, P, 2, IDXF], mybir.dt.int16, kind="ExternalInput"
    )
    rel_h = nc.dram_tensor("rel", [NT, P, K, DIM], F32, kind="ExternalInput")
    msk_h = nc.dram_tensor("msk", [NT, P, 2, GR, K], F32, kind="ExternalInput")
    out_h = nc.dram_tensor("score", [NT, P, K], F32, kind="ExternalOutput")

    NB = 2     # gather-buffer pipeline slots
    NB_IO = 4  # idx/rel/msk prefetch slots
    NOPS = 17  # DVE ops per tile

    with (
        nc.Block() as block,
        nc.sbuf_tensor("gbuf", [P, NB, 2, CPT, QC, GR * DIM], F32) as gbuf,
        nc.sbuf_tensor("relb", [P, NB_IO, K, DIM], F32) as relb,
        nc.sbuf_tensor("mskb", [P, NB_IO, 2, GR, K], F32) as mskb,
        nc.sbuf_tensor("idxb", [P, NB_IO, 2, IDXF], mybir.dt.int16) as idxb,
        nc.sbuf_tensor("hbuf", [P, K, DIM], F32) as hbuf,
        nc.sbuf_tensor("tbuf", [P, K, DIM], F32) as tbuf,
        nc.sbuf_tensor("tmpb", [P, K, DIM], F32) as tmpb,
        nc.sbuf_tensor("sbuf_s", [P, NB, K], F32) as sb_s,
        nc.semaphore("s_idx") as s_idx,
        nc.semaphore("s_rel") as s_rel,
        nc.semaphore("s_msk") as s_msk,
        nc.semaphore("s_g") as s_g,
        nc.semaphore("s_g2") as s_g2,
        nc.semaphore("s_g3") as s_g3,
        nc.semaphore("s_g4") as s_g4,
        nc.semaphore("s_vc") as s_vc,
        nc.semaphore("s_out") as s_out,
    ):

        @block.sync
        def _(sp: bass.BassEngine):
            # pure prefetcher: never gated on the compute chain beyond
            # slot reuse (NB_IO slots deep)
            for t in range(NT):
                s = t % NB_IO
                if t >= 1:
                    # order completions: sem count N must imply tiles 0..N-1
                    # are actually resident (DMAs can finish out of order)
                    sp.wait_ge(s_idx, 16 * t)
                    sp.wait_ge(s_rel, 16 * t)
                    sp.wait_ge(s_msk, 16 * t)
                if t >= NB_IO:
                    # idx slot free once tile t-NB_IO's gathers retired
                    tt = t - NB_IO
                    sp.wait_ge((s_g, s_g2)[tt % 2], 16 * (tt // 2 + 1))
                    sp.wait_ge((s_g3, s_g4)[tt % 2], 16 * (tt // 2 + 1))
                sp.dma_start(out=idxb[:, s], in_=idx_h[t]).then_inc(s_idx, 16)
                if t >= NB_IO:
                    # rel/msk slots consumed by DVE of tile t-NB_IO
                    sp.wait_ge(s_vc, NOPS * (t - NB_IO + 1))
                sp.dma_start(out=relb[:, s], in_=rel_h[t]).then_inc(s_rel, 16)
                sp.dma_start(out=mskb[:, s], in_=msk_h[t]).then_inc(s_msk, 16)

        @block.scalar
        def _(sc: bass.BassEngine):
            # out-stores, decoupled from the prefetch stream
            for t in range(NT):
                sc.wait_ge(s_vc, NOPS * (t + 1))
                if t >= 1:
                    sc.wait_ge(s_out, 16 * t)
                sc.dma_start(
                    out=out_h[t], in_=sb_s[:, t % NB]
                ).then_inc(s_out, 16)
            sc.wait_ge(s_out, 16 * NT)

        @block.gpsimd
        def _(gp: bass.BassGpSimd):
            gp.load_library(library_config.mlp)
            for t in range(NT):
                s = t % NB
                gp.wait_ge(s_idx, 16 * (t + 1))
                if t >= NB:
                    # gather buffers of tile t-NB consumed by DVE
                    gp.wait_ge(s_vc, NOPS * (t - NB + 1))
                for d in range(2):
                    gp.dma_gather(
                        gbuf[:, s, d, 0],
                        table[:],
                        idxb[:, t % NB_IO, d],
                        CHUNK,
                        _chunk_valid(t, 0),
                        GR * DIM,
                        elem_step=GR * DIM,
                        single_packet=False,
                    ).then_inc(
                        (s_g, s_g2)[t % 2] if d == 0 else (s_g3, s_g4)[t % 2],
                        16,
                    )

        @block.vector
        def _(v: bass.BassEngine):
            mult = mybir.AluOpType.mult
            add = mybir.AluOpType.add
            for t in range(NT):
                s = t % NB
                v.wait_ge((s_g, s_g2)[t % 2], 16 * (t // 2 + 1))  # src landed
                v.wait_ge(s_rel, 16 * (t + 1))
                v.wait_ge(s_msk, 16 * (t + 1))
                if t >= NB:
                    v.wait_ge(s_out, 16 * (t - NB + 1))
                if t >= 1:
                    # hbuf/tbuf/tmpb WAR vs previous tile's chain
                    v.wait_ge(s_vc, NOPS * t)
                i = NOPS * t

                def op(instr):
                    nonlocal i
                    i += 1
                    instr.then_inc(s_vc, 1)

                def wait():
                    v.wait_ge(s_vc, i)

                for d, dst in ((0, hbuf), (1, tbuf)):
                    if d == 1:
                        v.wait_ge((s_g3, s_g4)[t % 2], 16 * (t // 2 + 1))
                    # g viewed [P, K, GR*DIM]: (c, q) -> k = c*QC + q
                    g = gbuf[:, s, d].rearrange("p c q e -> p (c q) e")
                    for r in range(GR):
                        m = mskb[:, t % NB_IO, d, r].to_broadcast([P, K, DIM])
                        gsl = g[:, :, r * DIM : (r + 1) * DIM]
                        if r == 0:
                            op(v.tensor_tensor(out=dst[:], in0=gsl, in1=m, op=mult))
                        else:
                            wait()
                            op(
                                v.tensor_tensor(
                                    out=tmpb[:], in0=gsl, in1=m, op=mult
                                )
                            )
                            wait()
                            op(
                                v.tensor_tensor(
                                    out=dst[:], in0=dst[:], in1=tmpb[:], op=add
                                )
                            )
                wait()
                op(
                    v.tensor_tensor(
                        out=hbuf[:], in0=hbuf[:], in1=relb[:, t % NB_IO], op=mult
                    )
                )
                wait()
                op(
                    v.tensor_tensor(
                        out=hbuf[:], in0=hbuf[:], in1=tbuf[:], op=mult
                    )
                )
                wait()
                v.tensor_reduce(
                    out=sb_s[:, s],
                    in_=hbuf[:],
                    axis=mybir.AxisListType.X,
                    op=add,
                ).then_inc(s_vc, 1)

    nc.compile()
    _cache["nc"] = nc
    return nc


def _prep_idx(idx_global):
    """(EPAD,) node ids -> wrapped int16 group indices [NT, P, IDXF].
    Pad edges (index < 0) stay -1 and are skipped via num_idxs_reg."""
    n = idx_global.reshape(NT, TILE)
    g = np.where(n >= 0, n >> 2, -1).astype(np.int16)
    for t in range(NT):
        for c in range(CPT):
            if t * TILE + c * CHUNK >= EPC:
                g[t, c * CHUNK] = 0  # keep >=1 valid idx per chunk
    # wrap: index j -> [j % 16, j // 16], replicated across 8 partition groups
    wr = g.reshape(NT, IDXF, 16).swapaxes(1, 2)  # [NT, 16, IDXF]
    return np.broadcast_to(wr[:, None, :, :], (NT, 8, 16, IDXF)).reshape(
        NT, P, IDXF
    )


def _prep_msk(idx_global):
    """(EPAD,) node ids -> 0/1 row-select masks [NT, P, GR, K]."""
    sub = (np.maximum(idx_global.reshape(NT, K, P), 0) & 3).astype(np.int8)
    m = (sub[:, None, :, :] == np.arange(GR, dtype=np.int8)[None, :, None, None])
    # [NT, GR, K, P] -> [NT, P, GR, K]
    return np.ascontiguousarray(m.transpose(0, 3, 1, 2)).astype(np.float32)


def _shard_inputs(node_emb, rel_emb, src, dst):
    node_emb = np.asarray(node_emb, dtype=np.float32)
    rel_emb = np.asarray(rel_emb, dtype=np.float32)
    src = np.asarray(src, dtype=np.int64)
    dst = np.asarray(dst, dtype=np.int64)

    table = np.ascontiguousarray(node_emb.reshape(NGRP, GR * DIM))

    in_maps = []
    for c in range(N_CORES):
        sl = slice(c * EPC, (c + 1) * EPC)
        src_c = np.full(EPAD, -1, np.int64)
        dst_c = np.full(EPAD, -1, np.int64)
        rel_c = np.zeros((EPAD, DIM), np.float32)
        src_c[:EPC] = src[sl]
        dst_c[:EPC] = dst[sl]
        rel_c[:EPC] = rel_emb[sl]

        idx = np.stack([_prep_idx(src_c), _prep_idx(dst_c)], axis=2)
        # [NT, P, 2, IDXF]
        msk = np.stack([_prep_msk(src_c), _prep_msk(dst_c)], axis=2)
        # [NT, P, 2, GR, K]
        # edge j at [p = j % 128, k = j // 128] -> rel[t, p, k]
        rel_t = np.ascontiguousarray(
            rel_c.reshape(NT, K, P, DIM).swapaxes(1, 2)
        )
        in_maps.append(
            {
                "table": table,
                "idx": np.ascontiguousarray(idx),
                "rel": rel_t,
                "msk": msk,
            }
        )
    return in_maps


def run_on_hw(node_emb, rel_emb, src, dst, **spmd_kwargs):
    nc = _build_program()
    in_maps = _shard_inputs(node_emb, rel_emb, src, dst)
    res = run_bass_kernel_spmd(nc, in_maps, list(range(N_CORES)), **spmd_kwargs)
    parts = [
        np.asarray(res.results[c]["score"])
        .transpose(0, 2, 1)
        .reshape(EPAD)[:EPC]
        for c in range(N_CORES)
    ]
    return np.concatenate(parts), res


def kernel(node_emb, rel_emb, src, dst):
    scores, _ = run_on_hw(node_emb, rel_emb, src, dst)
    return scores


# revision 22
# speedup vs baseline: 1.1861x; 1.0029x over previous
"""DistMult edge scoring on 8 Trainium2 NeuronCores.

score[e] = sum_d node_emb[src[e], d] * rel_emb[e, d] * node_emb[dst[e], d]

Strategy (data-parallel over edges, per the sharding hint):
  - Edges sharded contiguously across 8 cores (125k/core, padded to whole
    128x32 tiles). Node table replicated per core in HBM.
  - Gather: gpsimd dma_gather is descriptor-count-bound (~8 ns/descriptor,
    measured), descriptor SIZE is nearly free, and int16 indices cap the
    addressable range at 32767. So the table is viewed as 25000 groups of
    4 rows (1 KB each): one descriptor fetches an edge's whole 4-row group
    (group id = node >> 2 < 25000 fits int16), and DVE selects the right
    row with host-precomputed 0/1 masks (node & 3). One descriptor per
    edge per direction - 4x fewer than any windowed row-gather scheme.
  - Per tile of 4096 edges: 2 gathers (one per direction, 4096 indices
    each in multi-packet mode), one rel load, DVE mask-select + product +
    segmented reduce, scores out. Measured: ~7.9 ns/descriptor, GpSimd
    descgen-saturated (~50 us idle in 2.1 ms).
  - Raw bacc with a manually pipelined 2-slot schedule (Tile does not
    semaphorize InstDMAGatherAnt).
"""

import numpy as np

import concourse.bacc as bacc
import concourse.bass as bass
import concourse.mybir as mybir
from concourse import library_config
from concourse.bass_utils import run_bass_kernel_spmd

N_NODES = 100000
DIM = 64
N_EDGES = 1000000
N_CORES = 8

P = 128
K = 32
TILE = P * K                      # 4096 edges per tile
EPC = N_EDGES // N_CORES          # 125000
NT = -(-EPC // TILE)              # 31
EPAD = NT * TILE                  # 126976

GR = 4                            # rows per gather group
NGRP = N_NODES // GR              # 25000 groups, fits int16
CHUNK = TILE                      # idx per dma_gather (multi-packet mode)
CPT = TILE // CHUNK               # 1 chunk per tile
QC = CHUNK // P                   # 32 free slots per chunk

IDXF = TILE // 16                 # 256 int16s per partition per direction
F32 = mybir.dt.float32

_cache = {}


def _chunk_valid(t, c):
    """Valid (non-pad) edge count in tile t, chunk c; pad idx are trailing
    -1s skipped by num_idxs_reg semantics. All-pad chunks keep one valid
    index (host forces slot 0 to group 0) so the DMA completion fires."""
    lo = t * TILE + c * CHUNK
    return int(max(1, min(CHUNK, EPC - lo)))


def _build_program():
    if "nc" in _cache:
        return _cache["nc"]

    nc = bacc.Bacc(
        "TRN2",
        target_bir_lowering=False,
        debug=False,
        enable_asserts=False,
        num_devices=N_CORES,
    )
    table = nc.dram_tensor("table", [NGRP, GR * DIM], F32, kind="ExternalInput")
    idx_h = nc.dram_tensor(
        "idx", [NT, P, 2, IDXF], mybir.dt.int16, kind="ExternalInput"
    )
    rel_h = nc.dram_tensor("rel", [NT, P, K, DIM], F32, kind="ExternalInput")
    msk_h = nc.dram_tensor("msk", [NT, P, 2, GR, K], F32, kind="ExternalInput")
    out_h = nc.dram_tensor("score", [NT, P, K], F32, kind="ExternalOutput")

    NB = 2     # gather-buffer pipeline slots
    NB_IO = 4  # idx/rel/msk prefetch slots
    NOPS = 17  # DVE ops per tile

    with (
        nc.Block() as block,
        nc.sbuf_tensor("gbuf", [P, NB, 2, CPT, QC, GR * DIM], F32) as gbuf,
        nc.sbuf_tensor("relb", [P, NB_IO, K, DIM], F32) as relb,
        nc.sbuf_tensor("mskb", [P, NB_IO, 2, GR, K], F32) as mskb,
        nc.sbuf_tensor("idxb", [P, NB_IO, 2, IDXF], mybir.dt.int16) as idxb,
        nc.sbuf_tensor("hbuf", [P, K, DIM], F32) as hbuf,
        nc.sbuf_tensor("tbuf", [P, K, DIM], F32) as tbuf,
        nc.sbuf_tensor("tmpb", [P, K, DIM], F32) as tmpb,
        nc.sbuf_tensor("sbuf_s", [P, NB, K], F32) as sb_s,
        nc.semaphore("s_idx") as s_idx,
        nc.semaphore("s_rel") as s_rel,
        nc.semaphore("s_msk") as s_msk,
        nc.semaphore("s_g") as s_g,
        nc.semaphore("s_g2") as s_g2,
        nc.semaphore("s_g3") as s_g3,
        nc.semaphore("s_g4") as s_g4,
        nc.semaphore("s_vc") as s_vc,
        nc.semaphore("s_out") as s_out,
    ):

        @block.sync
        def _(sp: bass.BassEngine):
            # pure prefetcher: never gated on the compute chain beyond
            # slot reuse (NB_IO slots deep)
            for t in range(NT):
                s = t % NB_IO
                if t >= 1:
                    # order completions: sem count N must imply tiles 0..N-1
                    # are actually resident (DMAs can finish out of order)
                    sp.wait_ge(s_idx, 16 * t)
                    sp.wait_ge(s_rel, 16 * t)
                    sp.wait_ge(s_msk, 16 * t)
                if t >= NB_IO:
                    # idx slot free once tile t-NB_IO's gathers retired
                    tt = t - NB_IO
                    sp.wait_ge((s_g, s_g2)[tt % 2], 16 * (tt // 2 + 1))
                    sp.wait_ge((s_g3, s_g4)[tt % 2], 16 * (tt // 2 + 1))
                sp.dma_start(out=idxb[:, s], in_=idx_h[t]).then_inc(s_idx, 16)
                if t >= NB_IO:
                    # rel/msk slots consumed by DVE of tile t-NB_IO
                    sp.wait_ge(s_vc, NOPS * (t - NB_IO + 1))
                sp.dma_start(out=relb[:, s], in_=rel_h[t]).then_inc(s_rel, 16)
                sp.dma_start(out=mskb[:, s], in_=msk_h[t]).then_inc(s_msk, 16)

        @block.scalar
        def _(sc: bass.BassEngine):
            # out-stores, decoupled from the prefetch stream
            for t in range(NT):
                sc.wait_ge(s_vc, NOPS * (t + 1))
                if t >= 1:
                    sc.wait_ge(s_out, 16 * t)
                sc.dma_start(
                    out=out_h[t], in_=sb_s[:, t % NB]
                ).then_inc(s_out, 16)
            sc.wait_ge(s_out, 16 * NT)

        @block.gpsimd
        def _(gp: bass.BassGpSimd):
            gp.load_library(library_config.mlp)
            for t in range(NT):
                s = t % NB
                gp.wait_ge(s_idx, 16 * (t + 1))
                if t >= NB:
                    # gather buffers of tile t-NB consumed by DVE
                    gp.wait_ge(s_vc, NOPS * (t - NB + 1))
                for d in range(2):
                    gp.dma_gather(
                        gbuf[:, s, d, 0],
                        table[:],
                        idxb[:, t % NB_IO, d],
                        CHUNK,
                        _chunk_valid(t, 0),
                        GR * DIM,
                        elem_step=GR * DIM,
                        single_packet=False,
                    ).then_inc(
                        (s_g, s_g2)[t % 2] if d == 0 else (s_g3, s_g4)[t % 2],
                        16,
                    )

        @block.vector
        def _(v: bass.BassEngine):
            mult = mybir.AluOpType.mult
            add = mybir.AluOpType.add
            for t in range(NT):
                s = t % NB
                v.wait_ge((s_g, s_g2)[t % 2], 16 * (t // 2 + 1))  # src landed
                v.wait_ge(s_rel, 16 * (t + 1))
                v.wait_ge(s_msk, 16 * (t + 1))
                if t >= NB:
                    v.wait_ge(s_out, 16 * (t - NB + 1))
                if t >= 1:
                    # hbuf/tbuf/tmpb WAR vs previous tile's chain
                    v.wait_ge(s_vc, NOPS * t)
                # last tile: only ceil(valid/P) k-slots hold real edges
                KV = K if t < NT - 1 else -(-(EPC - t * TILE) // P)
                i = NOPS * t

                def op(instr):
                    nonlocal i
                    i += 1
                    instr.then_inc(s_vc, 1)

                def wait():
                    v.wait_ge(s_vc, i)

                for d, dst in ((0, hbuf), (1, tbuf)):
                    if d == 1:
                        v.wait_ge((s_g3, s_g4)[t % 2], 16 * (t // 2 + 1))
                    # g viewed [P, K, GR*DIM]: (c, q) -> k = c*QC + q
                    g = gbuf[:, s, d].rearrange("p c q e -> p (c q) e")
                    for r in range(GR):
                        m = mskb[:, t % NB_IO, d, r, :KV].to_broadcast(
                            [P, KV, DIM]
                        )
                        gsl = g[:, :KV, r * DIM : (r + 1) * DIM]
                        if r == 0:
                            op(
                                v.tensor_tensor(
                                    out=dst[:, :KV], in0=gsl, in1=m, op=mult
                                )
                            )
                        else:
                            wait()
                            op(
                                v.tensor_tensor(
                                    out=tmpb[:, :KV], in0=gsl, in1=m, op=mult
                                )
                            )
                            wait()
                            op(
                                v.tensor_tensor(
                                    out=dst[:, :KV],
                                    in0=dst[:, :KV],
                                    in1=tmpb[:, :KV],
                                    op=add,
                                )
                            )
                wait()
                op(
                    v.tensor_tensor(
                        out=hbuf[:, :KV],
                        in0=hbuf[:, :KV],
                        in1=relb[:, t % NB_IO, :KV],
                        op=mult,
                    )
                )
                wait()
                op(
                    v.tensor_tensor(
                        out=hbuf[:, :KV], in0=hbuf[:, :KV], in1=tbuf[:, :KV],
                        op=mult,
                    )
                )
                wait()
                v.tensor_reduce(
                    out=sb_s[:, s, :KV],
                    in_=hbuf[:, :KV],
                    axis=mybir.AxisListType.X,
                    op=add,
                ).then_inc(s_vc, 1)

    nc.compile()
    _cache["nc"] = nc
    return nc


def _prep_idx(idx_global):
    """(EPAD,) node ids -> wrapped int16 group indices [NT, P, IDXF].
    Pad edges (index < 0) stay -1 and are skipped via num_idxs_reg."""
    n = idx_global.reshape(NT, TILE)
    g = np.where(n >= 0, n >> 2, -1).astype(np.int16)
    for t in range(NT):
        for c in range(CPT):
            if t * TILE + c * CHUNK >= EPC:
                g[t, c * CHUNK] = 0  # keep >=1 valid idx per chunk
    # wrap: index j -> [j % 16, j // 16], replicated across 8 partition groups
    wr = g.reshape(NT, IDXF, 16).swapaxes(1, 2)  # [NT, 16, IDXF]
    return np.broadcast_to(wr[:, None, :, :], (NT, 8, 16, IDXF)).reshape(
        NT, P, IDXF
    )


def _prep_msk(idx_global):
    """(EPAD,) node ids -> 0/1 row-select masks [NT, P, GR, K]."""
    sub = (np.maximum(idx_global.reshape(NT, K, P), 0) & 3).astype(np.int8)
    m = (sub[:, None, :, :] == np.arange(GR, dtype=np.int8)[None, :, None, None])
    # [NT, GR, K, P] -> [NT, P, GR, K]
    return np.ascontiguousarray(m.transpose(0, 3, 1, 2)).astype(np.float32)


def _shard_inputs(node_emb, rel_emb, src, dst):
    node_emb = np.asarray(node_emb, dtype=np.float32)
    rel_emb = np.asarray(rel_emb, dtype=np.float32)
    src = np.asarray(src, dtype=np.int64)
    dst = np.asarray(dst, dtype=np.int64)

    table = np.ascontiguousarray(node_emb.reshape(NGRP, GR * DIM))

    in_maps = []
    for c in range(N_CORES):
        sl = slice(c * EPC, (c + 1) * EPC)
        src_c = np.full(EPAD, -1, np.int64)
        dst_c = np.full(EPAD, -1, np.int64)
        rel_c = np.zeros((EPAD, DIM), np.float32)
        src_c[:EPC] = src[sl]
        dst_c[:EPC] = dst[sl]
        rel_c[:EPC] = rel_emb[sl]

        idx = np.stack([_prep_idx(src_c), _prep_idx(dst_c)], axis=2)
        # [NT, P, 2, IDXF]
        msk = np.stack([_prep_msk(src_c), _prep_msk(dst_c)], axis=2)
        # [NT, P, 2, GR, K]
        # edge j at [p = j % 128, k = j // 128] -> rel[t, p, k]
        rel_t = np.ascontiguousarray(
            rel_c.reshape(NT, K, P, DIM).swapaxes(1, 2)
        )
        in_maps.append(
            {
                "table": table,
                "idx": np.ascontiguousarray(idx),
                "rel": rel_t,
                "msk": msk,
            }
        )
    return in_maps


def run_on_hw(node_emb, rel_emb, src, dst, **spmd_kwargs):
    nc = _build_program()
    in_maps = _shard_inputs(node_emb, rel_emb, src, dst)
    res = run_bass_kernel_spmd(nc, in_maps, list(range(N_CORES)), **spmd_kwargs)
    parts = [
        np.asarray(res.results[c]["score"])
        .transpose(0, 2, 1)
        .reshape(EPAD)[:EPC]
        for c in range(N_CORES)
    ]
    return np.concatenate(parts), res


def kernel(node_emb, rel_emb, src, dst):
    scores, _ = run_on_hw(node_emb, rel_emb, src, dst)
    return scores
